# revision 1
# baseline (speedup 1.0000x reference)
"""Trainium2 Bass kernel for nn_DensePoseV1ConvXGNSparseGNHead.

Reference computation (per layer l in 0..7):
    y = x @ W[l] + b[l]
    per-instance GroupNorm over (tokens-of-instance, group-channels)
    x = xn * gamma[l] + beta[l]
    per-instance ECA: m = mean_tokens(x); conv1d(k=3) over channels; gate=sigmoid
    x = relu(x * gate[ids])

Strategy:
  * Host sorts points by instance id. 64 instances -> 8 cores x 8 slots.
    Each slot is padded to S columns (S >= max instance size). Zero
    cross-core communication is needed.
  * On-chip layout: x is [channel, point] = 2 blocks of 128 partitions,
    points along the free dim. Matmul per layer: lhsT = W chunk
    [c_in(128), c_out(128)], rhs = x chunk [c_in(128), cols], PSUM out
    [c_out(128), cols], accumulated over the 2 c_in chunks.
  * Per layer elementwise pipeline (fp16 data):
      - ScalarE copies PSUM->SBUF (cast fp16) with accum_out => per
        (channel, slot) raw sums s1 for free.
      - VectorE tensor_tensor_reduce(y*y) => raw sums of squares s2.
      - normalize+affine+gate+relu fused into ONE tensor_scalar pass via
          relu(y*A + B) = max(y*A, -B) + B
        The +B shift is NOT applied elementwise; it is carried
        analytically (per instance,channel) into the next layer:
          x_true = x' + B,  y_next = x'@W + (B@W),  corrections to the
        next layer's stats/bias are tiny [128, 8] tensors.
      - Padded columns hold a per-slot constant vector which is also
        tracked analytically and subtracted from raw stats.
  * All transcendentals via one ACT table set: rsqrt = exp(-0.5*ln(.)),
    sigmoid = 1/(1+exp(-z)) with VectorE reciprocal.

The host applies the final +B shift and un-sorts/un-pads the output.
"""

import os
import sys

sys.path.insert(0, "/opt/trn_rl_repo")

import numpy as np
import ml_dtypes

import concourse.bass as bass
import concourse.tile as tile
from concourse import bacc, mybir
from concourse._compat import with_exitstack  # noqa: F401

# ---------------------------------------------------------------- constants
N = 120000
C = 256
L = 8
G = 32          # groups
GS = C // G     # 8 channels per group
K = 3
NUM_INS = 64
EPS = 1e-5
NCORES = 8
IPC = NUM_INS // NCORES      # instances (slots) per core = 8
NBLK = 2                     # channel blocks of 128
P = 128

HALF = mybir.dt.float16
NP_HALF = np.float16
F32 = mybir.dt.float32

AF = mybir.ActivationFunctionType
OP = mybir.AluOpType

_PROGRAM_CACHE = {}
LAST_RESULTS = None   # test.py introspection


# ================================================================ device IR
def build_program(S: int, n_layers: int = L, use_accum: bool = True,
                  reps: int = 1, ablate: str = ''):
    """Build + compile the per-core Bass program for slot size S."""
    NC_COLS = IPC * S
    PFD = 1024                   # psum tile free size (2 banks)
    NH = -(-S // PFD)            # psum tiles per (slot, block)
    CHUNKS = [(h * PFD, min(PFD, S - h * PFD)) for h in range(NH)]

    nc = bacc.Bacc(
        "TRN2", target_bir_lowering=False, debug=False,
        enable_asserts=False, num_devices=NCORES,
    )

    # ---- DRAM I/O
    x0_d = nc.dram_tensor("x0", [NBLK, P, NC_COLS], HALF, kind="ExternalInput")
    w_d = nc.dram_tensor("wt", [P, L * NBLK * C], HALF, kind="ExternalInput")
    band_d = nc.dram_tensor("band", [P, L * NBLK * C], F32, kind="ExternalInput")
    gam_d = nc.dram_tensor("gam", [P, L * NBLK], F32, kind="ExternalInput")
    bet_d = nc.dram_tensor("bet", [P, L * NBLK], F32, kind="ExternalInput")
    bia_d = nc.dram_tensor("bia", [P, L * NBLK], F32, kind="ExternalInput")
    ggat_d = nc.dram_tensor("ggat", [P, G // NBLK], F32, kind="ExternalInput")
    gsca_d = nc.dram_tensor("gsca", [G // NBLK, P], F32, kind="ExternalInput")
    nvec_d = nc.dram_tensor("nvec", [P, IPC], F32, kind="ExternalInput")
    rn_d = nc.dram_tensor("rn", [P, IPC], F32, kind="ExternalInput")
    pc_d = nc.dram_tensor("pc", [P, IPC], F32, kind="ExternalInput")
    xout_d = nc.dram_tensor("xout", [NBLK, P, NC_COLS], F32, kind="ExternalOutput")
    bout_d = nc.dram_tensor("bout", [NBLK, P, IPC], F32, kind="ExternalOutput")

    GPB = G // NBLK  # groups per block = 16

    with tile.TileContext(nc) as tc:
        with (
            tc.tile_pool(name="persist", bufs=1) as persist,
            tc.tile_pool(name="xy", bufs=1) as xy,
            tc.tile_pool(name="ysq", bufs=3) as ysqp,
            tc.tile_pool(name="stage", bufs=2) as stage,
            tc.tile_pool(name="sm", bufs=4) as sm,
            tc.tile_pool(name="carry", bufs=2) as carry,
            tc.tile_pool(name="ps", bufs=2, space="PSUM") as psb,
            tc.tile_pool(name="pss", bufs=4, space="PSUM") as pss,
        ):
            # ---------- persistent constants
            w_sb = persist.tile([P, L * NBLK * C], HALF, tag="w")
            nc.sync.dma_start(w_sb[:], w_d.ap())
            band_sb = persist.tile([P, L * NBLK * C], F32, tag="band")
            nc.sync.dma_start(band_sb[:], band_d.ap())
            gam_sb = persist.tile([P, L * NBLK], F32, tag="gam")
            nc.sync.dma_start(gam_sb[:], gam_d.ap())
            bet_sb = persist.tile([P, L * NBLK], F32, tag="bet")
            nc.sync.dma_start(bet_sb[:], bet_d.ap())
            bia_sb = persist.tile([P, L * NBLK], F32, tag="bia")
            nc.sync.dma_start(bia_sb[:], bia_d.ap())
            ggat_sb = persist.tile([P, GPB], F32, tag="ggat")
            nc.sync.dma_start(ggat_sb[:], ggat_d.ap())
            gsca_sb = persist.tile([GPB, P], F32, tag="gsca")
            nc.sync.dma_start(gsca_sb[:], gsca_d.ap())
            nvec_sb = persist.tile([P, IPC], F32, tag="nvec")
            nc.sync.dma_start(nvec_sb[:], nvec_d.ap())
            rn_sb = persist.tile([P, IPC], F32, tag="rn")
            nc.sync.dma_start(rn_sb[:], rn_d.ap())
            pc_sb = persist.tile([P, IPC], F32, tag="pc")
            nc.sync.dma_start(pc_sb[:], pc_d.ap())
            eps_sb = persist.tile([P, 1], F32, tag="eps")
            nc.vector.memset(eps_sb[:], EPS)

            def wchunk(l, k, b):
                # lhsT [c_in(128) of chunk k, c_out 128b:128b+128]
                return w_sb[:, (l * NBLK + k) * C + b * P:(l * NBLK + k) * C + (b + 1) * P]

            def bandchunk(l, k, b):
                return band_sb[:, (l * NBLK + k) * C + b * P:(l * NBLK + k) * C + (b + 1) * P]

            # ---------- big data
            x_sb = [xy.tile([P, NC_COLS], HALF, tag=f"x{b}", name=f"xsb{b}") for b in range(NBLK)]
            y_sb = [xy.tile([P, NC_COLS], HALF, tag=f"y{b}", name=f"ysb{b}") for b in range(NBLK)]

            def one_run(rep):
                for b in range(NBLK):
                    for s in range(IPC):
                        nc.sync.dma_start(
                            x_sb[b][:, s * S:(s + 1) * S],
                            x0_d.ap()[b, :, s * S:(s + 1) * S],
                        )

                # ---------- cross-layer carried state
                v_prev = [carry.tile([P, IPC], HALF, tag=f"v{b}", name=f"vprev{rep}_{b}") for b in range(NBLK)]
                bq_prev = [carry.tile([P, IPC], HALF, tag=f"bq{b}", name=f"bqprev{rep}_{b}") for b in range(NBLK)]
                for b in range(NBLK):
                    nc.vector.memset(v_prev[b][:], 0.0)
                    nc.vector.memset(bq_prev[b][:], 0.0)

                for l in range(n_layers):
                    last = l == n_layers - 1
                    # ---- tiny PE matmuls: D = Bq_prev @ W, vp = v_prev @ W
                    d_ps = [pss.tile([P, IPC], F32, tag="psm", name=f"dps{rep}_{l}_{i}") for i in range(NBLK)]
                    vp_ps = [pss.tile([P, IPC], F32, tag="psm", name=f"vpps{rep}_{l}_{i}") for i in range(NBLK)]
                    for b in range(NBLK):
                        for k in range(NBLK):
                            nc.tensor.matmul(d_ps[b][:], wchunk(l, k, b), bq_prev[k][:],
                                             start=(k == 0), stop=(k == 1))
                        for k in range(NBLK):
                            nc.tensor.matmul(vp_ps[b][:], wchunk(l, k, b), v_prev[k][:],
                                             start=(k == 0), stop=(k == 1))
                    d_t = [sm.tile([P, IPC], F32, tag="d", name=f"dt{rep}_{l}_{i}") for i in range(NBLK)]
                    vpf = [sm.tile([P, IPC], F32, tag="vpf", name=f"vpf{rep}_{l}_{i}") for i in range(NBLK)]
                    vpb = [sm.tile([P, IPC], HALF, tag="vpb", name=f"vpb{rep}_{l}_{i}") for i in range(NBLK)]
                    for b in range(NBLK):
                        nc.vector.tensor_copy(d_t[b][:], d_ps[b][:])
                        nc.vector.tensor_copy(vpf[b][:], vp_ps[b][:])
                        nc.vector.tensor_copy(vpb[b][:], vp_ps[b][:])

                    # ---- main matmuls + PSUM->SBUF copy (w/ s1 accum) + sumsq
                    s1h = [sm.tile([P, IPC * NH], F32, tag="s1h", name=f"s1h{rep}_{l}_{i}") for i in range(NBLK)]
                    s2t = [sm.tile([P, IPC], F32, tag="s2t", name=f"s2t{rep}_{l}_{i}") for i in range(NBLK)]
                    for s in range(IPC):
                        for b in range(NBLK):
                            for h, (hoff, clen) in enumerate(CHUNKS):
                                pt = psb.tile([P, PFD], F32, tag="big")
                                c0 = s * S + hoff
                                for q0 in range(0, clen, 512):
                                    qn = min(512, clen - q0)
                                    for k in range(NBLK):
                                        nc.tensor.matmul(
                                            pt[:, q0:q0 + qn],
                                            wchunk(l, k, b),
                                            x_sb[k][:, c0 + q0:c0 + q0 + qn],
                                            start=(k == 0), stop=(k == 1),
                                        )
                                # ScalarE: copy+cast with accumulate (raw s1)
                                if use_accum:
                                    nc.scalar.activation(
                                        y_sb[b][:, c0:c0 + clen], pt[:, :clen],
                                        AF.Copy,
                                        accum_out=s1h[b][:, s * NH + h:s * NH + h + 1],
                                    )
                                else:
                                    nc.scalar.activation(
                                        y_sb[b][:, c0:c0 + clen], pt[:, :clen], AF.Copy)
                                    nc.vector.tensor_reduce(
                                        out=s1h[b][:, s * NH + h:s * NH + h + 1],
                                        in_=y_sb[b][:, c0:c0 + PFD],
                                        axis=mybir.AxisListType.X, op=OP.add)
                            # VectorE: sum of squares for the whole slot
                            # sum of squares: one DVE op per slot-block
                            ysq = ysqp.tile([P, S], HALF, tag="ysq",
                                            name=f"ysq{rep}_{l}_{s}_{b}")
                            nc.vector.scalar_tensor_tensor(
                                out=ysq[:],
                                in0=y_sb[b][:, s * S:(s + 1) * S],
                                scalar=1.0,
                                in1=y_sb[b][:, s * S:(s + 1) * S],
                                op0=OP.mult, op1=OP.mult,
                                accum_out=s2t[b][:, s:s + 1])

                    # ---- small-domain statistics -> A, negB
                    a_t = [sm.tile([P, IPC], F32, tag="a", name=f"at{rep}_{l}_{i}") for i in range(NBLK)]
                    nb_t = [sm.tile([P, IPC], F32, tag="nb", name=f"nbt{rep}_{l}_{i}") for i in range(NBLK)]
                    bq_new = [carry.tile([P, IPC], HALF, tag=f"bq{b}", name=f"bqn{rep}_{l}_{b}") for b in range(NBLK)]
                    v_new = [carry.tile([P, IPC], HALF, tag=f"v{b}", name=f"vn{rep}_{l}_{b}") for b in range(NBLK)]
                    maff = [sm.tile([P, IPC], F32, tag="maff", name=f"maff{rep}_{l}_{i}") for i in range(NBLK)]

                    for b in range(NBLK):
                        t0 = sm.tile([P, IPC], F32, tag="t0")
                        t1 = sm.tile([P, IPC], F32, tag="t1")
                        s1 = sm.tile([P, IPC], F32, tag="s1")
                        s2 = sm.tile([P, IPC], F32, tag="s2")
                        ee = sm.tile([P, IPC], F32, tag="ee")
                        # s1 = sum over halves - pc*vpf  (pad correction)
                        if NH == 2:
                            nc.vector.tensor_tensor(
                                out=s1[:], in0=s1h[b][:, 0::NH], in1=s1h[b][:, 1::NH],
                                op=OP.add)
                        else:
                            nc.vector.tensor_reduce(
                                out=s1[:],
                                in_=s1h[b][:].rearrange("p (s h) -> p s h", h=NH),
                                axis=mybir.AxisListType.X, op=OP.add)
                        nc.vector.tensor_tensor(out=t0[:], in0=pc_sb[:], in1=vpf[b][:],
                                                op=OP.mult)
                        nc.vector.tensor_tensor(out=s1[:], in0=s1[:], in1=t0[:],
                                                op=OP.subtract)
                        # s2 = s2t_raw - pc*vpb^2   (pad correction)
                        nc.vector.tensor_tensor(out=t0[:], in0=vpb[b][:], in1=vpb[b][:],
                                                op=OP.mult)
                        nc.vector.tensor_tensor(out=t0[:], in0=t0[:], in1=pc_sb[:],
                                                op=OP.mult)
                        nc.vector.tensor_tensor(out=s2[:], in0=s2t[b][:], in1=t0[:],
                                                op=OP.subtract)
                        # E = D + bias_c ;  S1 += n*E ; S2 += 2*E*s1 + n*E^2
                        nc.vector.tensor_scalar(
                            out=ee[:], in0=d_t[b][:],
                            scalar1=bia_sb[:, l * NBLK + b:l * NBLK + b + 1],
                            scalar2=None, op0=OP.add)
                        nc.vector.tensor_tensor(out=t0[:], in0=ee[:], in1=s1[:], op=OP.mult)
                        nc.vector.tensor_scalar(out=t0[:], in0=t0[:], scalar1=2.0,
                                                scalar2=None, op0=OP.mult)
                        nc.vector.tensor_tensor(out=t1[:], in0=ee[:], in1=ee[:], op=OP.mult)
                        nc.vector.tensor_tensor(out=t1[:], in0=t1[:], in1=nvec_sb[:],
                                                op=OP.mult)
                        nc.vector.tensor_tensor(out=t0[:], in0=t0[:], in1=t1[:], op=OP.add)
                        nc.vector.tensor_tensor(out=s2[:], in0=s2[:], in1=t0[:], op=OP.add)
                        nc.vector.tensor_tensor(out=t0[:], in0=nvec_sb[:], in1=ee[:],
                                                op=OP.mult)
                        nc.vector.tensor_tensor(out=s1[:], in0=s1[:], in1=t0[:], op=OP.add)

                        # group aggregation (PE one-hot)
                        g1_ps = pss.tile([P, IPC], F32, tag="psm")
                        g2_ps = pss.tile([P, IPC], F32, tag="psm")
                        nc.tensor.matmul(g1_ps[:GPB, :], ggat_sb[:], s1[:])
                        nc.tensor.matmul(g2_ps[:GPB, :], ggat_sb[:], s2[:])
                        gm = sm.tile([P, IPC], F32, tag="gm")
                        gq = sm.tile([P, IPC], F32, tag="gq")
                        # mu_g = Sg1 * rn / 8 ; E2_g = Sg2 * rn / 8 ; var = E2 - mu^2
                        nc.vector.tensor_tensor(out=gm[:GPB, :], in0=g1_ps[:GPB, :],
                                                in1=rn_sb[:GPB, :], op=OP.mult)
                        nc.vector.tensor_scalar(out=gm[:GPB, :], in0=gm[:GPB, :],
                                                scalar1=1.0 / GS, scalar2=None, op0=OP.mult)
                        nc.vector.tensor_tensor(out=gq[:GPB, :], in0=g2_ps[:GPB, :],
                                                in1=rn_sb[:GPB, :], op=OP.mult)
                        nc.vector.tensor_scalar(out=gq[:GPB, :], in0=gq[:GPB, :],
                                                scalar1=1.0 / GS, scalar2=None, op0=OP.mult)
                        gv = sm.tile([P, IPC], F32, tag="gv")
                        nc.vector.tensor_tensor(out=gv[:GPB, :], in0=gm[:GPB, :],
                                                in1=gm[:GPB, :], op=OP.mult)
                        nc.vector.tensor_tensor(out=gv[:GPB, :], in0=gq[:GPB, :],
                                                in1=gv[:GPB, :], op=OP.subtract)
                        # inv_g = rsqrt(var + eps): int-magic guess + 3 Newton
                        nc.vector.tensor_scalar(out=gv[:GPB, :], in0=gv[:GPB, :],
                                                scalar1=EPS, scalar2=None, op0=OP.add)
                        rs = sm.tile([P, IPC], F32, tag="rs", name=f"rs{rep}_{l}_{b}")
                        rt = sm.tile([P, IPC], F32, tag="rt", name=f"rt{rep}_{l}_{b}")
                        rsu = rs[:GPB, :].bitcast(mybir.dt.uint32)
                        nc.vector.tensor_scalar(
                            out=rsu, in0=gv[:GPB, :].bitcast(mybir.dt.uint32),
                            scalar1=1, scalar2=None, op0=OP.logical_shift_right)
                        nc.vector.tensor_scalar(out=rsu, in0=rsu, scalar1=0x7FFFFFFF,
                                                scalar2=None, op0=OP.bitwise_xor)
                        nc.vector.tensor_scalar(out=rsu, in0=rsu,
                                                scalar1=0x7FFFFFFF - 0x5F3759DF,
                                                scalar2=None, op0=OP.subtract)
                        for _ in range(3):
                            nc.vector.tensor_tensor(out=rt[:GPB, :], in0=rs[:GPB, :],
                                                    in1=rs[:GPB, :], op=OP.mult)
                            nc.vector.tensor_tensor(out=rt[:GPB, :], in0=rt[:GPB, :],
                                                    in1=gv[:GPB, :], op=OP.mult)
                            nc.vector.tensor_scalar(out=rt[:GPB, :], in0=rt[:GPB, :],
                                                    scalar1=-0.5, scalar2=1.5,
                                                    op0=OP.mult, op1=OP.add)
                            nc.vector.tensor_tensor(out=rs[:GPB, :], in0=rs[:GPB, :],
                                                    in1=rt[:GPB, :], op=OP.mult)
                        # broadcast to channels
                        iv_ps = pss.tile([P, IPC], F32, tag="psm")
                        mu_ps = pss.tile([P, IPC], F32, tag="psm")
                        nc.tensor.matmul(iv_ps[:], gsca_sb[:], rs[:GPB, :])
                        nc.tensor.matmul(mu_ps[:], gsca_sb[:], gm[:GPB, :])
                        iv = sm.tile([P, IPC], F32, tag="iv")
                        mu = sm.tile([P, IPC], F32, tag="mu")
                        nc.vector.tensor_copy(iv[:], iv_ps[:])
                        nc.vector.tensor_copy(mu[:], mu_ps[:])

                        # ECA mean (affine-transformed): ((S1*rn) - mu) * iv * gam + bet
                        nc.vector.tensor_tensor(out=t0[:], in0=s1[:], in1=rn_sb[:],
                                                op=OP.mult)
                        nc.vector.tensor_tensor(out=t0[:], in0=t0[:], in1=mu[:],
                                                op=OP.subtract)
                        nc.vector.tensor_tensor(out=t0[:], in0=t0[:], in1=iv[:],
                                                op=OP.mult)
                        nc.vector.tensor_scalar(
                            out=maff[b][:], in0=t0[:],
                            scalar1=gam_sb[:, l * NBLK + b:l * NBLK + b + 1],
                            scalar2=bet_sb[:, l * NBLK + b:l * NBLK + b + 1],
                            op0=OP.mult, op1=OP.add)
                        # stash iv, mu, ee for the second block pass
                        if b == 0:
                            iv0, mu0, ee0 = iv, mu, ee
                        else:
                            iv1, mu1, ee1 = iv, mu, ee

                    # ECA conv across channels (PE banded matmul) + sigmoid gate
                    cv_ps = [pss.tile([P, IPC], F32, tag="psm", name=f"cvps{rep}_{l}_{i}") for i in range(NBLK)]
                    for b in range(NBLK):
                        for k in range(NBLK):
                            nc.tensor.matmul(cv_ps[b][:], bandchunk(l, k, b), maff[k][:],
                                             start=(k == 0), stop=(k == 1))
                    for b in range(NBLK):
                        iv, mu, ee = (iv0, mu0, ee0) if b == 0 else (iv1, mu1, ee1)
                        gate = sm.tile([P, IPC], F32, tag="gate")
                        nc.scalar.activation(gate[:], cv_ps[b][:], AF.Sigmoid,
                                             bias=0.0, scale=1.0)
                        # A = iv * gam * gate
                        t0 = sm.tile([P, IPC], F32, tag="t0")
                        nc.vector.tensor_tensor(out=t0[:], in0=iv[:], in1=gate[:],
                                                op=OP.mult)
                        nc.vector.tensor_scalar(
                            out=a_t[b][:], in0=t0[:],
                            scalar1=gam_sb[:, l * NBLK + b:l * NBLK + b + 1],
                            scalar2=None, op0=OP.mult)
                        # B = ((E - mu) * iv * gam + bet) * gate ; quantize to bf16
                        t1 = sm.tile([P, IPC], F32, tag="t1")
                        nc.vector.tensor_tensor(out=t1[:], in0=ee[:], in1=mu[:],
                                                op=OP.subtract)
                        nc.vector.tensor_tensor(out=t1[:], in0=t1[:], in1=iv[:],
                                                op=OP.mult)
                        nc.vector.tensor_scalar(
                            out=t1[:], in0=t1[:],
                            scalar1=gam_sb[:, l * NBLK + b:l * NBLK + b + 1],
                            scalar2=bet_sb[:, l * NBLK + b:l * NBLK + b + 1],
                            op0=OP.mult, op1=OP.add)
                        nc.vector.tensor_tensor(out=t1[:], in0=t1[:], in1=gate[:],
                                                op=OP.mult)
                        nc.vector.tensor_copy(bq_new[b][:], t1[:])          # bf16
                        nc.vector.tensor_scalar(out=nb_t[b][:], in0=bq_new[b][:],
                                                scalar1=-1.0, scalar2=None, op0=OP.mult)
                        # v_next = max(vpb * A, -Bq)   (matches padded columns)
                        t2 = sm.tile([P, IPC], F32, tag="t2")
                        nc.vector.tensor_tensor(out=t2[:], in0=vpb[b][:], in1=a_t[b][:],
                                                op=OP.mult)
                        nc.vector.tensor_tensor(out=v_new[b][:], in0=t2[:], in1=nb_t[b][:],
                                                op=OP.max)
                        if last:
                            nc.sync.dma_start(bout_d.ap()[b], nb_t[b][:])

                    # ---- fused normalize+gate+relu:  x' = max(y'*A, -Bq)
                    for s in range(IPC):
                        for b in range(NBLK):
                            if not last and "norm" in ablate:
                                pass
                            elif not last:
                                nc.vector.tensor_scalar(
                                    out=x_sb[b][:, s * S:(s + 1) * S],
                                    in0=y_sb[b][:, s * S:(s + 1) * S],
                                    scalar1=a_t[b][:, s:s + 1],
                                    scalar2=nb_t[b][:, s:s + 1],
                                    op0=OP.mult, op1=OP.max)
                            else:
                                ot = stage.tile([P, S], F32, tag="out")
                                nc.vector.tensor_scalar(
                                    out=ot[:],
                                    in0=y_sb[b][:, s * S:(s + 1) * S],
                                    scalar1=a_t[b][:, s:s + 1],
                                    scalar2=nb_t[b][:, s:s + 1],
                                    op0=OP.mult, op1=OP.max)
                                nc.sync.dma_start(
                                    xout_d.ap()[b, :, s * S:(s + 1) * S], ot[:])

                    v_prev, bq_prev = v_new, bq_new

            for rep in range(reps):
                one_run(rep)

    nc.compile()
    return nc


# ================================================================ host side
def _prepare(features, W, b, gamma, beta, eca_w, ins_indices):
    counts = np.bincount(ins_indices, minlength=NUM_INS).astype(np.int64)
    order = np.argsort(ins_indices, kind="stable")
    starts = np.zeros(NUM_INS + 1, np.int64)
    np.cumsum(counts, out=starts[1:])

    S = int(max(1088, -(-int(counts.max()) // 64) * 64))
    NC_COLS = IPC * S

    feat_s = np.ascontiguousarray(features[order].T)        # [C, N] sorted
    feat_bf = feat_s.astype(NP_HALF)

    x0 = np.zeros((NCORES, NBLK, P, NC_COLS), NP_HALF)
    nvec = np.zeros((NCORES, P, IPC), np.float32)
    rn = np.zeros((NCORES, P, IPC), np.float32)
    pc = np.zeros((NCORES, P, IPC), np.float32)
    for c in range(NCORES):
        for i in range(IPC):
            g = c * IPC + i
            n = int(counts[g])
            if n:
                x0[c, 0, :, i * S:i * S + n] = feat_bf[:P, starts[g]:starts[g] + n]
                x0[c, 1, :, i * S:i * S + n] = feat_bf[P:, starts[g]:starts[g] + n]
            nvec[c, :, i] = float(n)
            rn[c, :, i] = 1.0 / float(max(n, 1))
            pc[c, :, i] = float(S - n)

    # [P, (l k c)]: wt[p, (l,k,c)] = W[l][k*128+p, c]
    wt = np.ascontiguousarray(
        np.asarray(W, np.float32).reshape(L, NBLK, P, C)
        .transpose(2, 0, 1, 3).reshape(P, L * NBLK * C)).astype(NP_HALF)

    # conv[c'] = sum_j w_j * m[c'+j-1]  =>  T[cin, cout]=w_j with cout=cin-j+1
    band = np.zeros((L, C, C), np.float32)
    idx = np.arange(C)
    for j in range(K):
        d = K // 2 - j   # cout - cin
        cin = idx[(idx + d >= 0) & (idx + d < C)]
        band[:, cin, cin + d] = np.asarray(eca_w, np.float32)[:, j][:, None]
    band = np.ascontiguousarray(
        band.reshape(L, NBLK, P, C).transpose(2, 0, 1, 3).reshape(P, L * NBLK * C))

    def _perchan(a):  # [L, C] -> [P, L*NBLK]
        return np.ascontiguousarray(
            np.asarray(a, np.float32).reshape(L, NBLK, P).transpose(2, 0, 1)
            .reshape(P, L * NBLK))
    gam = _perchan(gamma)
    bet = _perchan(beta)
    bia = _perchan(b)

    gpb = G // NBLK
    ggat = np.zeros((P, gpb), np.float32)
    ggat[np.arange(P), np.arange(P) // GS] = 1.0
    gsca = np.ascontiguousarray(ggat.T)

    shared = dict(wt=wt, band=band, gam=gam, bet=bet, bia=bia,
                  ggat=ggat, gsca=gsca)
    in_maps = []
    for c in range(NCORES):
        m = dict(shared)
        m.update(x0=np.ascontiguousarray(x0[c]), nvec=np.ascontiguousarray(nvec[c]),
                 rn=np.ascontiguousarray(rn[c]), pc=np.ascontiguousarray(pc[c]))
        in_maps.append(m)
    return in_maps, counts, order, starts, S


def _assemble(results, counts, order, starts, S, n_points=N):
    out = np.empty((n_points, C), np.float32)
    for c in range(NCORES):
        xo = results[c]["xout"]          # [2, 128, NC_COLS] fp32 (= x')
        nb = results[c]["bout"]          # [2, 128, IPC] fp32 (= -Bq)
        for i in range(IPC):
            g = c * IPC + i
            n = int(counts[g])
            if n == 0:
                continue
            blk = xo[:, :, i * S:i * S + n] - nb[:, :, i:i + 1]
            out[order[starts[g]:starts[g] + n], :] = (
                blk.reshape(C, n).T)
    return out


def kernel(features, W, b, gamma, beta, eca_w, ins_indices):
    global LAST_RESULTS
    features = np.asarray(features, np.float32)
    W = np.asarray(W, np.float32)
    b = np.asarray(b, np.float32)
    gamma = np.asarray(gamma, np.float32)
    beta = np.asarray(beta, np.float32)
    eca_w = np.asarray(eca_w, np.float32)
    ins_indices = np.asarray(ins_indices, np.int32)

    in_maps, counts, order, starts, S = _prepare(
        features, W, b, gamma, beta, eca_w, ins_indices)

    if S not in _PROGRAM_CACHE:
        _PROGRAM_CACHE[S] = build_program(S)
    nc = _PROGRAM_CACHE[S]

    from concourse import bass_utils
    res = bass_utils.run_bass_kernel_spmd(
        nc, in_maps, core_ids=list(range(NCORES)), trace=False)
    LAST_RESULTS = res
    return _assemble(res.results, counts, order, starts, S,
                     n_points=features.shape[0])




